# revision 7
# baseline (speedup 1.0000x reference)
"""GAT NodeEncoder kernel for Trainium2 (8 NeuronCores, data-parallel over batch).

Reference computation (per batch element b, per node n):
    src  = E[subgraph[b,n]];  nei_i = E[neighs[b,n,i]]
    s_0  = leaky(src@a1 + src@a2 + a_b); s_i = leaky(src@a1 + nei_i@a2 + a_b) + mask_i*-1e9
    att  = softmax(s); v = sum_i att_i * emb_i
    x = leaky(fc1 @ [v; local_stats; gstat] + b1); out = leaky(fc2 @ x + b2)

Sharding: batch B=8 over 8 cores (1 batch row / core), emb table replicated
(uploaded bf16 -- matches the bf16 in-flight compute precision).

Gather strategy (the SWDGE fixed cost of ~1us/call dominates a naive
128-rows-per-indirect-DMA approach):
  Phase 1: per core, the ~21k UNIQUE referenced table rows are gathered with
  4 dma_gather ucode calls (one per 32768-row window -- dma_gather indices
  are int16) into SBUF, then flushed per-window to a DRAM scratch laid out
  so that scratch rows are addressable by an int16 rank.
  Phase 2: per 128-node tile, ONE dma_gather from the scratch fetches all
  (node, slot) rows positionally (slot-major) -- ~3000 rows per call at
  994ns + 0.34ns/row of gpsimd time.

Compute per tile (slots uniform per tile via degree-sorting, masked
neighbors dropped exactly):
  scores   = reduce_X(g * a2_bcast) on DVE; s = Lrelu(w + (u+ab)) on ACT
  softmax  = max/exp+accum/recip; att = Copy(e, scale=1/z) on ACT
  weighted = g * att (stride-0-inner broadcast) + strided reduce on DVE
  head     = PE transpose via identity, fc1/fc2 on PE, Lrelu+bias on ACT
Output rows are stored directly (HWDGE) in sorted order; host unsorts.
"""

import os
from contextlib import ExitStack

import numpy as np
import ml_dtypes

import concourse.bass as bass
import concourse.bacc as bacc
import concourse.tile as tile
from concourse import mybir
from concourse import bass_utils
from concourse import library_config

B, S, N, H, NLS = 8, 1024, 32, 128, 4
NUM_NODES = 100001
TILE = 128
NT = S // TILE
WIN = 32768
NWIN = 4
F32 = mybir.dt.float32
BF16 = mybir.dt.bfloat16
I32 = mybir.dt.int32
I16 = mybir.dt.int16
AF = mybir.ActivationFunctionType
ALU = mybir.AluOpType

_cached = {}


def _rup(x, m):
    return (x + m - 1) // m * m


def _build_program(slots, ni_ws, v_ws):
    """slots: per-tile slot counts; ni_ws/v_ws: per-window phase-1 static
    num_idxs (mult of 16) and valid counts (SPMD-uniform)."""
    nt = len(slots)
    ctot = int(sum(slots))
    offs = np.concatenate([[0], np.cumsum(slots)]).astype(int)
    blocks_w = [_rup(ni, TILE) // TILE for ni in ni_ws]
    blk_off = np.concatenate([[0], np.cumsum(blocks_w)]).astype(int)
    blk_tot = int(blk_off[-1])
    srows = TILE * blk_tot
    n1cols = [ni // 16 for ni in ni_ws]
    c1off = np.concatenate([[0], np.cumsum(n1cols)]).astype(int)
    n2cols = [TILE * int(c) // 16 for c in slots]
    c2off = np.concatenate([[0], np.cumsum(n2cols)]).astype(int)

    nc = bacc.Bacc(target_bir_lowering=False, debug=False, enable_asserts=False)

    emb = nc.dram_tensor("emb", [NUM_NODES, H], BF16, kind="ExternalInput")
    idx1 = nc.dram_tensor("idx1", [TILE, int(c1off[-1])], I16, kind="ExternalInput")
    idx2 = nc.dram_tensor("idx2", [TILE, int(c2off[-1])], I16, kind="ExternalInput")
    padm = nc.dram_tensor("padm", [TILE, ctot], F32, kind="ExternalInput")
    statst = nc.dram_tensor("statst", [NLS + 1, S], BF16, kind="ExternalInput")
    a2rep_d = nc.dram_tensor("a2rep", [1, H], BF16, kind="ExternalInput")
    a1rep_d = nc.dram_tensor("a1rep", [1, H], BF16, kind="ExternalInput")
    ab_rep = nc.dram_tensor("ab_rep", [TILE, 1], F32, kind="ExternalInput")
    ident = nc.dram_tensor("ident", [TILE, TILE], BF16, kind="ExternalInput")
    w1t_a = nc.dram_tensor("w1t_a", [H, H], BF16, kind="ExternalInput")
    w1t_b = nc.dram_tensor("w1t_b", [NLS + 1, H], BF16, kind="ExternalInput")
    b1 = nc.dram_tensor("b1", [H, 1], F32, kind="ExternalInput")
    w2t = nc.dram_tensor("w2t", [H, H], BF16, kind="ExternalInput")
    b2row = nc.dram_tensor("b2row", [1, H], BF16, kind="ExternalInput")
    onesc = nc.dram_tensor("onesc", [1, TILE], BF16, kind="ExternalInput")
    out = nc.dram_tensor("out", [S, H], F32, kind="ExternalOutput")

    cmax = int(max(slots))

    with tile.TileContext(nc) as tc, ExitStack() as ctx:
        dpool = ctx.enter_context(tc.tile_pool(name="dram", bufs=1, space="DRAM"))
        const = ctx.enter_context(tc.tile_pool(name="const", bufs=1))
        gpool = ctx.enter_context(tc.tile_pool(name="gpool", bufs=2))
        spool = ctx.enter_context(tc.tile_pool(name="spool", bufs=2))
        small = ctx.enter_context(tc.tile_pool(name="small", bufs=4))
        opool = ctx.enter_context(tc.tile_pool(name="opool", bufs=2))
        psum = ctx.enter_context(tc.tile_pool(name="psum", bufs=2, space="PSUM"))

        scratch = dpool.tile([srows, H], BF16)

        # ---- constants ----
        c_idx1 = const.tile([TILE, int(c1off[-1])], I16)
        nc.sync.dma_start(out=c_idx1[:], in_=idx1[:, :])
        c_idx2_0 = const.tile([TILE, int(c2off[-1])], I16)
        nc.sync.dma_start(out=c_idx2_0[:], in_=idx2[:, :])
        c_padm0 = const.tile([TILE, ctot], F32)
        nc.sync.dma_start(out=c_padm0[:], in_=padm[:, :])
        c_stats = const.tile([NLS + 1, S], BF16)
        nc.sync.dma_start(out=c_stats[:], in_=statst[:, :])
        c_ab = const.tile([TILE, 1], F32)
        nc.sync.dma_start(out=c_ab[:], in_=ab_rep[:, :])
        c_id = const.tile([TILE, TILE], BF16)
        nc.sync.dma_start(out=c_id[:], in_=ident[:, :])
        c_w1a = const.tile([H, H], BF16)
        nc.sync.dma_start(out=c_w1a[:], in_=w1t_a[:, :])
        c_w1b = const.tile([NLS + 1, H], BF16)
        nc.sync.dma_start(out=c_w1b[:], in_=w1t_b[:, :])
        c_b1 = const.tile([H, 1], F32)
        nc.sync.dma_start(out=c_b1[:], in_=b1[:, :])
        c_w2 = const.tile([H, H], BF16)
        nc.sync.dma_start(out=c_w2[:], in_=w2t[:, :])
        c_b2 = const.tile([1, H], BF16)
        nc.sync.dma_start(out=c_b2[:], in_=b2row[:, :])
        c_ones = const.tile([1, TILE], BF16)
        nc.sync.dma_start(out=c_ones[:], in_=onesc[:, :])
        # a1/a2 replicated to 128 partitions (DMA broadcast)
        c_a2r0 = const.tile([TILE, H], BF16)
        nc.gpsimd.dma_start(out=c_a2r0[:], in_=bass.AP(
            tensor=a2rep_d.ap().tensor, offset=0, ap=[[0, TILE], [1, H]]))
        c_a1r0 = const.tile([TILE, H], BF16)
        nc.gpsimd.dma_start(out=c_a1r0[:], in_=bass.AP(
            tensor=a1rep_d.ap().tensor, offset=0, ap=[[0, TILE], [1, H]]))

        nc.gpsimd.load_library(library_config.mlp)

        # ---- phase 1: gather unique rows per 32768-row window ----
        g1 = const.tile([TILE, blk_tot * H], BF16)
        for w in range(NWIN):
            if ni_ws[w] == 0:
                continue
            span = min(WIN, NUM_NODES - w * WIN)
            src_ap = bass.AP(tensor=emb.ap().tensor, offset=w * WIN * H,
                             ap=[[H, span], [1, H]])
            nc.gpsimd.dma_gather(
                g1[:, int(blk_off[w]) * H:int(blk_off[w + 1]) * H].rearrange(
                    "p (b h) -> p b h", b=blocks_w[w]),
                src_ap,
                c_idx1[:, int(c1off[w]):int(c1off[w + 1])],
                int(ni_ws[w]),
                int(v_ws[w]),
                H,
                single_packet=False,
            )
            # flush window w to scratch rows p*blk_tot + blk_off[w] + u
            nc.sync.dma_start(
                out=bass.AP(
                    tensor=scratch[:].tensor,
                    offset=scratch[:].offset + int(blk_off[w]) * H,
                    ap=[[blk_tot * H, TILE], [H, blocks_w[w]], [1, H]]),
                in_=g1[:, int(blk_off[w]) * H:int(blk_off[w + 1]) * H])

        # ---- one-time fences: absorb const-DMA sems onto consuming engines ----
        c_idx2 = const.tile([TILE, int(c2off[-1])], I16)
        nc.vector.tensor_copy(out=c_idx2[:], in_=c_idx2_0[:])
        c_a2r = const.tile([TILE, H], BF16)
        nc.vector.tensor_copy(out=c_a2r[:], in_=c_a2r0[:])
        c_a1r = const.tile([TILE, H], BF16)
        nc.vector.tensor_copy(out=c_a1r[:], in_=c_a1r0[:])
        c_padm = const.tile([TILE, ctot], F32)
        nc.vector.tensor_copy(out=c_padm[:], in_=c_padm0[:])
        c_ab2 = const.tile([TILE, 1], F32)
        nc.vector.tensor_copy(out=c_ab2[:], in_=c_ab[:])
        dpsum = psum.tile([TILE, TILE], F32, tag="dfence")
        nc.tensor.matmul(out=dpsum[:], lhsT=c_id[:], rhs=c_w1a[:], start=True, stop=True)
        nc.tensor.matmul(out=dpsum[:], lhsT=c_w2[:], rhs=c_id[:], start=True, stop=True)
        nc.tensor.matmul(
            out=dpsum[:], lhsT=c_w1b[:], rhs=c_stats[:, 0:TILE], start=True, stop=True)
        nc.tensor.matmul(out=dpsum[:], lhsT=c_ones[:], rhs=c_b2[:], start=True, stop=True)
        dact = const.tile([TILE, 1], F32)
        nc.scalar.activation(out=dact[:], in_=c_ab2[:], func=AF.Identity, bias=c_b1[:, 0:1])

        # ---- phase 2: per-tile positional gather + compute ----
        for t in range(nt):
            ct = int(slots[t])
            o0 = int(offs[t])
            g = gpool.tile([TILE, cmax * H], BF16, tag="g")
            nc.gpsimd.dma_gather(
                g[:, :ct * H].rearrange("p (i h) -> p i h", i=ct),
                scratch[:],
                c_idx2[:, int(c2off[t]):int(c2off[t + 1])],
                TILE * ct,
                TILE * ct,
                H,
                single_packet=False,
            )

            # ---- scores: w[:, i] = g_i . a2  (broadcast mul + seg reduce) ----
            t1 = spool.tile([TILE, cmax * H], BF16, tag="t1")
            a2b = bass.AP(tensor=c_a2r[:].tensor, offset=c_a2r[:].offset,
                          ap=[c_a2r[:].ap[0], [0, ct], [1, H]])
            nc.vector.tensor_tensor(
                out=t1[:, :ct * H].rearrange("p (i h) -> p i h", i=ct),
                in0=g[:, :ct * H].rearrange("p (i h) -> p i h", i=ct),
                in1=a2b, op=ALU.mult)
            w = small.tile([TILE, cmax], F32, tag="w")
            nc.vector.reduce_sum(
                out=w[:, :ct],
                in_=t1[:, :ct * H].rearrange("p (i h) -> p i h", i=ct),
                axis=mybir.AxisListType.X)
            # u = src . a1 (slot 0), then u' = u + a_b
            t2 = small.tile([TILE, H], BF16, tag="t2")
            nc.vector.tensor_tensor(out=t2[:], in0=g[:, :H], in1=c_a1r[:], op=ALU.mult)
            u = small.tile([TILE, 1], F32, tag="u")
            nc.vector.reduce_sum(
                out=u[:], in_=t2[:].rearrange("p (i h) -> p i h", i=1),
                axis=mybir.AxisListType.X)
            up = small.tile([TILE, 1], F32, tag="up")
            nc.vector.tensor_scalar(
                out=up[:], in0=u[:], scalar1=c_ab2[:, 0:1], scalar2=None,
                op0=ALU.add)
            # s = leaky(w + u'), then -1e9 on pad slots
            s0 = small.tile([TILE, cmax], F32, tag="s0")
            nc.scalar.activation(
                out=s0[:, :ct], in_=w[:, :ct], func=AF.Identity, bias=up[:, 0:1])
            s = small.tile([TILE, cmax], F32, tag="s")
            nc.vector.scalar_tensor_tensor(
                out=s[:, :ct], in0=s0[:, :ct], scalar=0.2, in1=s0[:, :ct],
                op0=ALU.mult, op1=ALU.max)
            nc.vector.scalar_tensor_tensor(
                out=s[:, :ct], in0=c_padm[:, o0:o0 + ct], scalar=-1e9,
                in1=s[:, :ct], op0=ALU.mult, op1=ALU.add)
            # softmax
            negm = small.tile([TILE, 1], F32, tag="negm")
            nc.vector.tensor_reduce(
                out=negm[:], in_=s[:, :ct], axis=mybir.AxisListType.X, op=ALU.max,
                negate=True)
            e = small.tile([TILE, cmax], F32, tag="e")
            zsum = small.tile([TILE, 1], F32, tag="zsum")
            nc.scalar.activation(
                out=e[:, :ct], in_=s[:, :ct], func=AF.Exp, bias=negm[:, 0:1],
                accum_out=zsum[:])
            r = small.tile([TILE, 1], F32, tag="r")
            nc.vector.reciprocal(out=r[:], in_=zsum[:])
            att = small.tile([TILE, cmax], F32, tag="att")
            nc.scalar.activation(
                out=att[:, :ct], in_=e[:, :ct], func=AF.Copy, scale=r[:, 0:1])

            # ---- weighted sum: gs = g * att (stride-0-inner bcast), vsum ----
            gs = spool.tile([TILE, cmax * H], BF16, tag="gs")
            attb = bass.AP(tensor=att[:].tensor, offset=att[:].offset,
                           ap=[att[:].ap[0], [1, ct], [0, H]])
            nc.vector.tensor_tensor(
                out=gs[:, :ct * H].rearrange("p (i h) -> p i h", i=ct),
                in0=g[:, :ct * H].rearrange("p (i h) -> p i h", i=ct),
                in1=attb, op=ALU.mult)
            v = small.tile([TILE, H], F32, tag="v")
            nc.vector.reduce_sum(
                out=v[:],
                in_=gs[:, :ct * H].rearrange("p (i h) -> p h i", i=ct),
                axis=mybir.AxisListType.X)
            vb = small.tile([TILE, H], BF16, tag="vb")
            nc.scalar.activation(out=vb[:], in_=v[:], func=AF.Copy)

            # ---- transpose v via PE identity ----
            vps = psum.tile([H, TILE], F32, tag="vps")
            nc.tensor.matmul(out=vps[:], lhsT=vb[:], rhs=c_id[:], start=True, stop=True)
            vt = small.tile([H, TILE], BF16, tag="vt")
            nc.scalar.activation(out=vt[:], in_=vps[:], func=AF.Copy)

            # ---- MLP head ----
            o1p = psum.tile([H, TILE], F32, tag="o1p")
            nc.tensor.matmul(out=o1p[:], lhsT=c_w1a[:], rhs=vt[:], start=True, stop=False)
            nc.tensor.matmul(
                out=o1p[:], lhsT=c_w1b[:], rhs=c_stats[:, t * TILE:(t + 1) * TILE],
                start=False, stop=True)
            o1c = small.tile([H, TILE], BF16, tag="o1c")
            nc.scalar.activation(out=o1c[:], in_=o1p[:], func=AF.Identity, bias=c_b1[:, 0:1])
            o1 = small.tile([H, TILE], BF16, tag="o1")
            nc.vector.scalar_tensor_tensor(
                out=o1[:], in0=o1c[:], scalar=0.2, in1=o1c[:], op0=ALU.mult, op1=ALU.max)
            o2p = psum.tile([TILE, H], F32, tag="o2p")
            nc.tensor.matmul(out=o2p[:], lhsT=o1[:], rhs=c_w2[:], start=True, stop=False)
            nc.tensor.matmul(out=o2p[:], lhsT=c_ones[:], rhs=c_b2[:], start=False, stop=True)
            otc = small.tile([TILE, H], F32, tag="otc")
            nc.scalar.activation(out=otc[:], in_=o2p[:], func=AF.Copy)
            ot = opool.tile([TILE, H], F32, tag="ot")
            nc.vector.scalar_tensor_tensor(
                out=ot[:], in0=otc[:], scalar=0.2, in1=otc[:], op0=ALU.mult, op1=ALU.max)
            # direct store in sorted order (host unsorts)
            nc.sync.dma_start(
                out=bass.AP(tensor=out.ap().tensor, offset=t * TILE * H,
                            ap=[[H, TILE], [1, H]]),
                in_=ot[:])

    nc.finalize()
    return nc


def _prep_inputs(subgraph, neighs, mask, local_stats, global_stats,
                 emb_table, a_w, a_b, fc1_w, fc1_b, fc2_w, fc2_b):
    """Host-side layout/sharding prep.

    Returns (in_maps, orders, key) where key = (slots, ni_ws, v_ws)."""
    bf = ml_dtypes.bfloat16
    a1 = a_w[0, :H]
    a2 = a_w[0, H:]
    shared = {
        "emb": np.ascontiguousarray(emb_table).astype(bf),
        "a2rep": a2.reshape(1, H).astype(bf),
        "a1rep": a1.reshape(1, H).astype(bf),
        "ab_rep": np.broadcast_to(a_b.astype(np.float32), (TILE, 1)).copy(),
        "ident": np.eye(TILE, dtype=np.float32).astype(bf),
        "w1t_a": np.ascontiguousarray(fc1_w[:, :H].T).astype(bf),
        "w1t_b": np.ascontiguousarray(fc1_w[:, H:].T).astype(bf),
        "b1": fc1_b.reshape(H, 1).astype(np.float32),
        "w2t": np.ascontiguousarray(fc2_w.T).astype(bf),
        "b2row": fc2_b.reshape(1, H).astype(bf),
        "onesc": np.ones((1, TILE), dtype=np.float32).astype(bf),
    }
    keep = mask[:, :, :, 0] < 0.5          # [B,S,N] True = neighbor survives
    counts = 1 + keep.sum(axis=2)          # [B,S]
    orders = np.argsort(-counts, axis=1, kind="stable")

    slots = []
    for t in range(NT):
        c = 0
        for b in range(B):
            c = max(c, int(counts[b, orders[b, t * TILE]]))
        slots.append(c)
    slots = tuple(slots)
    offs = np.concatenate([[0], np.cumsum(slots)]).astype(int)
    ctot = int(offs[-1])

    # absolute slot ids per core (-1 = pad)
    idx_abs_all, padm_all, uniq_all = [], [], []
    for b in range(B):
        order = orders[b]
        idx_abs = np.full((TILE, ctot), -1, dtype=np.int64)
        padm = np.zeros((TILE, ctot), dtype=np.float32)
        for t in range(NT):
            ct = slots[t]
            o0 = offs[t]
            nodes = order[t * TILE:(t + 1) * TILE]
            idx_abs[:, o0] = subgraph[b, nodes]
            for p in range(TILE):
                n = nodes[p]
                kn = neighs[b, n][keep[b, n]]
                idx_abs[p, o0 + 1:o0 + 1 + len(kn)] = kn
                padm[p, o0 + 1 + len(kn):o0 + ct] = 1.0
        idx_abs_all.append(idx_abs)
        padm_all.append(padm)
        uniq_all.append(np.unique(idx_abs[idx_abs >= 0]))

    # per-window unique counts; SPMD-uniform valid counts
    wstarts = []
    for b in range(B):
        ws = np.searchsorted(uniq_all[b], np.arange(NWIN + 1) * WIN)
        wstarts.append(ws)
    v_ws = tuple(int(max(wstarts[b][w + 1] - wstarts[b][w] for b in range(B)))
                 for w in range(NWIN))
    ni_ws = tuple(_rup(v, 16) for v in v_ws)
    blocks_w = [_rup(ni, TILE) // TILE for ni in ni_ws]
    blk_off = np.concatenate([[0], np.cumsum(blocks_w)]).astype(int)
    blk_tot = int(blk_off[-1])
    assert TILE * blk_tot < 32768, f"scratch rows {TILE * blk_tot} exceed int16"

    def wrap16(a):
        return np.ascontiguousarray(a.reshape(-1, 16).T)

    in_maps = []
    for b in range(B):
        uniq = uniq_all[b]
        ws = wstarts[b]
        # phase-1 window index lists
        idx1_parts = []
        row_of_rank = np.empty(len(uniq), dtype=np.int64)
        for w in range(NWIN):
            if ni_ws[w] == 0:
                continue
            rel = uniq[ws[w]:ws[w + 1]] - w * WIN
            nwb = len(rel)
            a = np.full(ni_ws[w], -1, dtype=np.int16)
            a[:nwb] = rel.astype(np.int16)
            a[nwb:v_ws[w]] = 0                     # top-up (dup row) for SPMD
            idx1_parts.append(wrap16(a))
            j = np.arange(nwb)
            row_of_rank[ws[w]:ws[w + 1]] = (j % TILE) * blk_tot + blk_off[w] + j // TILE
        idx1 = np.tile(np.concatenate(idx1_parts, axis=1), (8, 1))

        # phase-2 positional rank lists (slot-major per tile)
        idx_abs = idx_abs_all[b]
        ranks = np.searchsorted(uniq, np.clip(idx_abs, 0, None))
        row2 = row_of_rank[ranks]
        row2[idx_abs < 0] = row_of_rank[0]         # pads -> a known-valid row
        assert row2.max() < 32768
        idx2_parts = []
        for t in range(NT):
            ct = slots[t]
            o0 = offs[t]
            arr = np.ascontiguousarray(row2[:, o0:o0 + ct].T).ravel()  # j = i*128+p
            idx2_parts.append(wrap16(arr.astype(np.int16)))
        idx2 = np.tile(np.concatenate(idx2_parts, axis=1), (8, 1))

        order = orders[b]
        st = np.concatenate(
            [local_stats[b][order].T,
             np.broadcast_to(global_stats[b].reshape(1, 1), (1, S))], axis=0)
        m = dict(shared)
        m.update({
            "idx1": idx1, "idx2": idx2, "padm": padm_all[b],
            "statst": np.ascontiguousarray(st).astype(bf),
        })
        in_maps.append(m)
    return in_maps, orders, (slots, ni_ws, v_ws)


last_exec_ns = None
last_results = None


def kernel(**inputs) -> np.ndarray:
    global last_exec_ns, last_results
    in_maps, orders, key = _prep_inputs(**inputs)
    if key not in _cached:
        _cached[key] = _build_program(*key)
    nc = _cached[key]
    trace = bool(int(os.environ.get("KERNEL_TRACE", "0")))
    res = bass_utils.run_bass_kernel_spmd(
        nc, in_maps, core_ids=list(range(B)), trace=trace)
    last_exec_ns = res.exec_time_ns
    last_results = res
    out = np.empty((B, S, H), dtype=np.float32)
    for b in range(B):
        out[b, orders[b]] = res.results[b]["out"]
    return out


if __name__ == "__main__":
    _build_program((33,) * NT, (5504, 5504, 5504, 448), (5500, 5500, 5500, 440))
    print("program builds OK")


# revision 8
# speedup vs baseline: 1.4192x; 1.4192x over previous
"""GAT NodeEncoder kernel for Trainium2 (8 NeuronCores, data-parallel over batch).

Reference computation (per batch element b, per node n):
    src  = E[subgraph[b,n]];  nei_i = E[neighs[b,n,i]]
    s_0  = leaky(src@a1 + src@a2 + a_b); s_i = leaky(src@a1 + nei_i@a2 + a_b) + mask_i*-1e9
    att  = softmax(s); v = sum_i att_i * emb_i
    x = leaky(fc1 @ [v; local_stats; gstat] + b1); out = leaky(fc2 @ x + b2)

Sharding: batch B=8 over 8 cores (1 batch row / core), emb table replicated
(uploaded bf16 -- matches the bf16 in-flight compute precision).

Gather strategy (the SWDGE fixed cost of ~1us/call dominates a naive
128-rows-per-indirect-DMA approach):
  Phase 1: per core, the ~21k UNIQUE referenced table rows are gathered with
  4 dma_gather ucode calls (one per 32768-row window -- dma_gather indices
  are int16) into SBUF, then flushed per-window to a DRAM scratch laid out
  so that scratch rows are addressable by an int16 rank.
  Phase 2: per 128-node tile, ONE dma_gather from the scratch fetches all
  (node, slot) rows positionally (slot-major) -- ~3000 rows per call at
  994ns + 0.34ns/row of gpsimd time.

Compute per tile (slots uniform per tile via degree-sorting, masked
neighbors dropped exactly):
  scores   = reduce_X(g * a2_bcast) on DVE; s = Lrelu(w + (u+ab)) on ACT
  softmax  = max/exp+accum/recip; att = Copy(e, scale=1/z) on ACT
  weighted = g * att (stride-0-inner broadcast) + strided reduce on DVE
  head     = PE transpose via identity, fc1/fc2 on PE, Lrelu+bias on ACT
Output rows are stored directly (HWDGE) in sorted order; host unsorts.
"""

import os
from contextlib import ExitStack

import numpy as np
import ml_dtypes

import concourse.bass as bass
import concourse.bacc as bacc
import concourse.tile as tile
from concourse import mybir
from concourse import bass_utils
from concourse import library_config

B, S, N, H, NLS = 8, 1024, 32, 128, 4
NUM_NODES = 100001
TILE = 128
NT = S // TILE
WIN = 32768
NWIN = 4
F32 = mybir.dt.float32
BF16 = mybir.dt.bfloat16
I32 = mybir.dt.int32
I16 = mybir.dt.int16
AF = mybir.ActivationFunctionType
ALU = mybir.AluOpType

_cached = {}


def _rup(x, m):
    return (x + m - 1) // m * m


def _build_program(slots, ni_ws, v_ws):
    """slots: per-tile slot counts; ni_ws/v_ws: per-window phase-1 static
    num_idxs (mult of 16) and valid counts (SPMD-uniform)."""
    nt = len(slots)
    ctot = int(sum(slots))
    offs = np.concatenate([[0], np.cumsum(slots)]).astype(int)
    blocks_w = [_rup(ni, TILE) // TILE for ni in ni_ws]
    blk_off = np.concatenate([[0], np.cumsum(blocks_w)]).astype(int)
    blk_tot = int(blk_off[-1])
    srows = TILE * blk_tot
    n1cols = [ni // 16 for ni in ni_ws]
    c1off = np.concatenate([[0], np.cumsum(n1cols)]).astype(int)
    n2cols = [TILE * int(c) // 16 for c in slots]
    c2off = np.concatenate([[0], np.cumsum(n2cols)]).astype(int)

    nc = bacc.Bacc(target_bir_lowering=False, debug=False, enable_asserts=False,
                   num_swdge_queues=4)

    emb = nc.dram_tensor("emb", [NUM_NODES, H], BF16, kind="ExternalInput")
    idx1 = nc.dram_tensor("idx1", [TILE, int(c1off[-1])], I16, kind="ExternalInput")
    idx2 = nc.dram_tensor("idx2", [TILE, int(c2off[-1])], I16, kind="ExternalInput")
    padm = nc.dram_tensor("padm", [TILE, ctot], F32, kind="ExternalInput")
    statst = nc.dram_tensor("statst", [NLS + 1, S], BF16, kind="ExternalInput")
    a2rep_d = nc.dram_tensor("a2rep", [1, H], BF16, kind="ExternalInput")
    a1rep_d = nc.dram_tensor("a1rep", [1, H], BF16, kind="ExternalInput")
    ab_rep = nc.dram_tensor("ab_rep", [TILE, 1], F32, kind="ExternalInput")
    ident = nc.dram_tensor("ident", [TILE, TILE], BF16, kind="ExternalInput")
    w1t_a = nc.dram_tensor("w1t_a", [H, H], BF16, kind="ExternalInput")
    w1t_b = nc.dram_tensor("w1t_b", [NLS + 1, H], BF16, kind="ExternalInput")
    b1 = nc.dram_tensor("b1", [H, 1], F32, kind="ExternalInput")
    w2t = nc.dram_tensor("w2t", [H, H], BF16, kind="ExternalInput")
    b2row = nc.dram_tensor("b2row", [1, H], BF16, kind="ExternalInput")
    onesc = nc.dram_tensor("onesc", [1, TILE], BF16, kind="ExternalInput")
    out = nc.dram_tensor("out", [S, H], F32, kind="ExternalOutput")

    cmax = int(max(slots))

    with tile.TileContext(nc) as tc, ExitStack() as ctx:
        dpool = ctx.enter_context(tc.tile_pool(name="dram", bufs=1, space="DRAM"))
        const = ctx.enter_context(tc.tile_pool(name="const", bufs=1))
        gpool = ctx.enter_context(tc.tile_pool(name="gpool", bufs=2))
        spool = ctx.enter_context(tc.tile_pool(name="spool", bufs=2))
        small = ctx.enter_context(tc.tile_pool(name="small", bufs=4))
        opool = ctx.enter_context(tc.tile_pool(name="opool", bufs=2))
        psum = ctx.enter_context(tc.tile_pool(name="psum", bufs=2, space="PSUM"))

        scratch = dpool.tile([srows, H], BF16)

        # ---- constants ----
        c_idx1 = const.tile([TILE, int(c1off[-1])], I16)
        nc.sync.dma_start(out=c_idx1[:], in_=idx1[:, :])
        c_idx2_0 = const.tile([TILE, int(c2off[-1])], I16)
        nc.sync.dma_start(out=c_idx2_0[:], in_=idx2[:, :])
        c_padm0 = const.tile([TILE, ctot], F32)
        nc.sync.dma_start(out=c_padm0[:], in_=padm[:, :])
        c_stats = const.tile([NLS + 1, S], BF16)
        nc.sync.dma_start(out=c_stats[:], in_=statst[:, :])
        c_ab = const.tile([TILE, 1], F32)
        nc.sync.dma_start(out=c_ab[:], in_=ab_rep[:, :])
        c_id = const.tile([TILE, TILE], BF16)
        nc.sync.dma_start(out=c_id[:], in_=ident[:, :])
        c_w1a = const.tile([H, H], BF16)
        nc.sync.dma_start(out=c_w1a[:], in_=w1t_a[:, :])
        c_w1b = const.tile([NLS + 1, H], BF16)
        nc.sync.dma_start(out=c_w1b[:], in_=w1t_b[:, :])
        c_b1 = const.tile([H, 1], F32)
        nc.sync.dma_start(out=c_b1[:], in_=b1[:, :])
        c_w2 = const.tile([H, H], BF16)
        nc.sync.dma_start(out=c_w2[:], in_=w2t[:, :])
        c_b2 = const.tile([1, H], BF16)
        nc.sync.dma_start(out=c_b2[:], in_=b2row[:, :])
        c_ones = const.tile([1, TILE], BF16)
        nc.sync.dma_start(out=c_ones[:], in_=onesc[:, :])
        # a1/a2 replicated to 128 partitions (DMA broadcast)
        c_a2r0 = const.tile([TILE, H], BF16)
        nc.gpsimd.dma_start(out=c_a2r0[:], in_=bass.AP(
            tensor=a2rep_d.ap().tensor, offset=0, ap=[[0, TILE], [1, H]]))
        c_a1r0 = const.tile([TILE, H], BF16)
        nc.gpsimd.dma_start(out=c_a1r0[:], in_=bass.AP(
            tensor=a1rep_d.ap().tensor, offset=0, ap=[[0, TILE], [1, H]]))

        nc.gpsimd.load_library(library_config.mlp)

        # ---- phase 1: gather unique rows per 32768-row window ----
        g1 = const.tile([TILE, blk_tot * H], BF16)
        for w in range(NWIN):
            if ni_ws[w] == 0:
                continue
            span = min(WIN, NUM_NODES - w * WIN)
            src_ap = bass.AP(tensor=emb.ap().tensor, offset=w * WIN * H,
                             ap=[[H, span], [1, H]])
            nc.gpsimd.dma_gather(
                g1[:, int(blk_off[w]) * H:int(blk_off[w + 1]) * H].rearrange(
                    "p (b h) -> p b h", b=blocks_w[w]),
                src_ap,
                c_idx1[:, int(c1off[w]):int(c1off[w + 1])],
                int(ni_ws[w]),
                int(v_ws[w]),
                H,
                single_packet=False,
                queue_num=w,
            )
            # flush window w to scratch rows p*blk_tot + blk_off[w] + u
            nc.sync.dma_start(
                out=bass.AP(
                    tensor=scratch[:].tensor,
                    offset=scratch[:].offset + int(blk_off[w]) * H,
                    ap=[[blk_tot * H, TILE], [H, blocks_w[w]], [1, H]]),
                in_=g1[:, int(blk_off[w]) * H:int(blk_off[w + 1]) * H])

        # ---- one-time fences: absorb const-DMA sems onto consuming engines ----
        c_idx2 = const.tile([TILE, int(c2off[-1])], I16)
        nc.vector.tensor_copy(out=c_idx2[:], in_=c_idx2_0[:])
        c_a2r = const.tile([TILE, H], BF16)
        nc.vector.tensor_copy(out=c_a2r[:], in_=c_a2r0[:])
        c_a1r = const.tile([TILE, H], BF16)
        nc.vector.tensor_copy(out=c_a1r[:], in_=c_a1r0[:])
        c_padm = const.tile([TILE, ctot], F32)
        nc.vector.tensor_copy(out=c_padm[:], in_=c_padm0[:])
        c_ab2 = const.tile([TILE, 1], F32)
        nc.vector.tensor_copy(out=c_ab2[:], in_=c_ab[:])
        dpsum = psum.tile([TILE, TILE], F32, tag="dfence")
        nc.tensor.matmul(out=dpsum[:], lhsT=c_id[:], rhs=c_w1a[:], start=True, stop=True)
        nc.tensor.matmul(out=dpsum[:], lhsT=c_w2[:], rhs=c_id[:], start=True, stop=True)
        nc.tensor.matmul(
            out=dpsum[:], lhsT=c_w1b[:], rhs=c_stats[:, 0:TILE], start=True, stop=True)
        nc.tensor.matmul(out=dpsum[:], lhsT=c_ones[:], rhs=c_b2[:], start=True, stop=True)
        dact = const.tile([TILE, 1], F32)
        nc.scalar.activation(out=dact[:], in_=c_ab2[:], func=AF.Identity, bias=c_b1[:, 0:1])

        # ---- phase 2: per-tile positional gather + compute ----
        for t in range(nt):
            ct = int(slots[t])
            o0 = int(offs[t])
            g = gpool.tile([TILE, cmax * H], BF16, tag="g")
            nc.gpsimd.dma_gather(
                g[:, :ct * H].rearrange("p (i h) -> p i h", i=ct),
                scratch[:],
                c_idx2[:, int(c2off[t]):int(c2off[t + 1])],
                TILE * ct,
                TILE * ct,
                H,
                single_packet=False,
                queue_num=t % 4,
            )

            # ---- scores: w[:, i] = g_i . a2  (broadcast mul + seg reduce) ----
            t1 = spool.tile([TILE, cmax * H], BF16, tag="t1")
            a2b = bass.AP(tensor=c_a2r[:].tensor, offset=c_a2r[:].offset,
                          ap=[c_a2r[:].ap[0], [0, ct], [1, H]])
            nc.vector.tensor_tensor(
                out=t1[:, :ct * H].rearrange("p (i h) -> p i h", i=ct),
                in0=g[:, :ct * H].rearrange("p (i h) -> p i h", i=ct),
                in1=a2b, op=ALU.mult)
            w = small.tile([TILE, cmax], F32, tag="w")
            nc.vector.reduce_sum(
                out=w[:, :ct],
                in_=t1[:, :ct * H].rearrange("p (i h) -> p i h", i=ct),
                axis=mybir.AxisListType.X)
            # u = src . a1 (slot 0), then u' = u + a_b
            t2 = small.tile([TILE, H], BF16, tag="t2")
            nc.vector.tensor_tensor(out=t2[:], in0=g[:, :H], in1=c_a1r[:], op=ALU.mult)
            u = small.tile([TILE, 1], F32, tag="u")
            nc.vector.reduce_sum(
                out=u[:], in_=t2[:].rearrange("p (i h) -> p i h", i=1),
                axis=mybir.AxisListType.X)
            up = small.tile([TILE, 1], F32, tag="up")
            nc.vector.tensor_scalar(
                out=up[:], in0=u[:], scalar1=c_ab2[:, 0:1], scalar2=None,
                op0=ALU.add)
            # s = leaky(w + u'), then -1e9 on pad slots
            s0 = small.tile([TILE, cmax], F32, tag="s0")
            nc.scalar.activation(
                out=s0[:, :ct], in_=w[:, :ct], func=AF.Identity, bias=up[:, 0:1])
            s = small.tile([TILE, cmax], F32, tag="s")
            nc.vector.scalar_tensor_tensor(
                out=s[:, :ct], in0=s0[:, :ct], scalar=0.2, in1=s0[:, :ct],
                op0=ALU.mult, op1=ALU.max)
            nc.vector.scalar_tensor_tensor(
                out=s[:, :ct], in0=c_padm[:, o0:o0 + ct], scalar=-1e9,
                in1=s[:, :ct], op0=ALU.mult, op1=ALU.add)
            # softmax
            negm = small.tile([TILE, 1], F32, tag="negm")
            nc.vector.tensor_reduce(
                out=negm[:], in_=s[:, :ct], axis=mybir.AxisListType.X, op=ALU.max,
                negate=True)
            e = small.tile([TILE, cmax], F32, tag="e")
            zsum = small.tile([TILE, 1], F32, tag="zsum")
            nc.scalar.activation(
                out=e[:, :ct], in_=s[:, :ct], func=AF.Exp, bias=negm[:, 0:1],
                accum_out=zsum[:])
            r = small.tile([TILE, 1], F32, tag="r")
            nc.vector.reciprocal(out=r[:], in_=zsum[:])
            att = small.tile([TILE, cmax], F32, tag="att")
            nc.scalar.activation(
                out=att[:, :ct], in_=e[:, :ct], func=AF.Copy, scale=r[:, 0:1])

            # ---- weighted sum: gs = g * att (stride-0-inner bcast), vsum ----
            gs = spool.tile([TILE, cmax * H], BF16, tag="gs")
            attb = bass.AP(tensor=att[:].tensor, offset=att[:].offset,
                           ap=[att[:].ap[0], [1, ct], [0, H]])
            nc.vector.tensor_tensor(
                out=gs[:, :ct * H].rearrange("p (i h) -> p i h", i=ct),
                in0=g[:, :ct * H].rearrange("p (i h) -> p i h", i=ct),
                in1=attb, op=ALU.mult)
            v = small.tile([TILE, H], F32, tag="v")
            nc.vector.reduce_sum(
                out=v[:],
                in_=gs[:, :ct * H].rearrange("p (i h) -> p h i", i=ct),
                axis=mybir.AxisListType.X)
            vb = small.tile([TILE, H], BF16, tag="vb")
            nc.scalar.activation(out=vb[:], in_=v[:], func=AF.Copy)

            # ---- transpose v via PE identity ----
            vps = psum.tile([H, TILE], F32, tag="vps")
            nc.tensor.matmul(out=vps[:], lhsT=vb[:], rhs=c_id[:], start=True, stop=True)
            vt = small.tile([H, TILE], BF16, tag="vt")
            nc.scalar.activation(out=vt[:], in_=vps[:], func=AF.Copy)

            # ---- MLP head ----
            o1p = psum.tile([H, TILE], F32, tag="o1p")
            nc.tensor.matmul(out=o1p[:], lhsT=c_w1a[:], rhs=vt[:], start=True, stop=False)
            nc.tensor.matmul(
                out=o1p[:], lhsT=c_w1b[:], rhs=c_stats[:, t * TILE:(t + 1) * TILE],
                start=False, stop=True)
            o1c = small.tile([H, TILE], BF16, tag="o1c")
            nc.scalar.activation(out=o1c[:], in_=o1p[:], func=AF.Identity, bias=c_b1[:, 0:1])
            o1 = small.tile([H, TILE], BF16, tag="o1")
            nc.vector.scalar_tensor_tensor(
                out=o1[:], in0=o1c[:], scalar=0.2, in1=o1c[:], op0=ALU.mult, op1=ALU.max)
            o2p = psum.tile([TILE, H], F32, tag="o2p")
            nc.tensor.matmul(out=o2p[:], lhsT=o1[:], rhs=c_w2[:], start=True, stop=False)
            nc.tensor.matmul(out=o2p[:], lhsT=c_ones[:], rhs=c_b2[:], start=False, stop=True)
            otc = small.tile([TILE, H], F32, tag="otc")
            nc.scalar.activation(out=otc[:], in_=o2p[:], func=AF.Copy)
            ot = opool.tile([TILE, H], F32, tag="ot")
            nc.vector.scalar_tensor_tensor(
                out=ot[:], in0=otc[:], scalar=0.2, in1=otc[:], op0=ALU.mult, op1=ALU.max)
            # direct store in sorted order (host unsorts)
            nc.sync.dma_start(
                out=bass.AP(tensor=out.ap().tensor, offset=t * TILE * H,
                            ap=[[H, TILE], [1, H]]),
                in_=ot[:])

    nc.finalize()
    return nc


def _prep_inputs(subgraph, neighs, mask, local_stats, global_stats,
                 emb_table, a_w, a_b, fc1_w, fc1_b, fc2_w, fc2_b):
    """Host-side layout/sharding prep.

    Returns (in_maps, orders, key) where key = (slots, ni_ws, v_ws)."""
    bf = ml_dtypes.bfloat16
    a1 = a_w[0, :H]
    a2 = a_w[0, H:]
    shared = {
        "emb": np.ascontiguousarray(emb_table).astype(bf),
        "a2rep": a2.reshape(1, H).astype(bf),
        "a1rep": a1.reshape(1, H).astype(bf),
        "ab_rep": np.broadcast_to(a_b.astype(np.float32), (TILE, 1)).copy(),
        "ident": np.eye(TILE, dtype=np.float32).astype(bf),
        "w1t_a": np.ascontiguousarray(fc1_w[:, :H].T).astype(bf),
        "w1t_b": np.ascontiguousarray(fc1_w[:, H:].T).astype(bf),
        "b1": fc1_b.reshape(H, 1).astype(np.float32),
        "w2t": np.ascontiguousarray(fc2_w.T).astype(bf),
        "b2row": fc2_b.reshape(1, H).astype(bf),
        "onesc": np.ones((1, TILE), dtype=np.float32).astype(bf),
    }
    keep = mask[:, :, :, 0] < 0.5          # [B,S,N] True = neighbor survives
    counts = 1 + keep.sum(axis=2)          # [B,S]
    orders = np.argsort(-counts, axis=1, kind="stable")

    slots = []
    for t in range(NT):
        c = 0
        for b in range(B):
            c = max(c, int(counts[b, orders[b, t * TILE]]))
        slots.append(c)
    slots = tuple(slots)
    offs = np.concatenate([[0], np.cumsum(slots)]).astype(int)
    ctot = int(offs[-1])

    # absolute slot ids per core (-1 = pad)
    idx_abs_all, padm_all, uniq_all = [], [], []
    for b in range(B):
        order = orders[b]
        idx_abs = np.full((TILE, ctot), -1, dtype=np.int64)
        padm = np.zeros((TILE, ctot), dtype=np.float32)
        for t in range(NT):
            ct = slots[t]
            o0 = offs[t]
            nodes = order[t * TILE:(t + 1) * TILE]
            idx_abs[:, o0] = subgraph[b, nodes]
            for p in range(TILE):
                n = nodes[p]
                kn = neighs[b, n][keep[b, n]]
                idx_abs[p, o0 + 1:o0 + 1 + len(kn)] = kn
                padm[p, o0 + 1 + len(kn):o0 + ct] = 1.0
        idx_abs_all.append(idx_abs)
        padm_all.append(padm)
        uniq_all.append(np.unique(idx_abs[idx_abs >= 0]))

    # per-window unique counts; SPMD-uniform valid counts
    wstarts = []
    for b in range(B):
        ws = np.searchsorted(uniq_all[b], np.arange(NWIN + 1) * WIN)
        wstarts.append(ws)
    v_ws = tuple(int(max(wstarts[b][w + 1] - wstarts[b][w] for b in range(B)))
                 for w in range(NWIN))
    ni_ws = tuple(_rup(v, 16) for v in v_ws)
    blocks_w = [_rup(ni, TILE) // TILE for ni in ni_ws]
    blk_off = np.concatenate([[0], np.cumsum(blocks_w)]).astype(int)
    blk_tot = int(blk_off[-1])
    assert TILE * blk_tot < 32768, f"scratch rows {TILE * blk_tot} exceed int16"

    def wrap16(a):
        return np.ascontiguousarray(a.reshape(-1, 16).T)

    in_maps = []
    for b in range(B):
        uniq = uniq_all[b]
        ws = wstarts[b]
        # phase-1 window index lists
        idx1_parts = []
        row_of_rank = np.empty(len(uniq), dtype=np.int64)
        for w in range(NWIN):
            if ni_ws[w] == 0:
                continue
            rel = uniq[ws[w]:ws[w + 1]] - w * WIN
            nwb = len(rel)
            a = np.full(ni_ws[w], -1, dtype=np.int16)
            a[:nwb] = rel.astype(np.int16)
            a[nwb:v_ws[w]] = 0                     # top-up (dup row) for SPMD
            idx1_parts.append(wrap16(a))
            j = np.arange(nwb)
            row_of_rank[ws[w]:ws[w + 1]] = (j % TILE) * blk_tot + blk_off[w] + j // TILE
        idx1 = np.tile(np.concatenate(idx1_parts, axis=1), (8, 1))

        # phase-2 positional rank lists (slot-major per tile)
        idx_abs = idx_abs_all[b]
        ranks = np.searchsorted(uniq, np.clip(idx_abs, 0, None))
        row2 = row_of_rank[ranks]
        row2[idx_abs < 0] = row_of_rank[0]         # pads -> a known-valid row
        assert row2.max() < 32768
        idx2_parts = []
        for t in range(NT):
            ct = slots[t]
            o0 = offs[t]
            arr = np.ascontiguousarray(row2[:, o0:o0 + ct].T).ravel()  # j = i*128+p
            idx2_parts.append(wrap16(arr.astype(np.int16)))
        idx2 = np.tile(np.concatenate(idx2_parts, axis=1), (8, 1))

        order = orders[b]
        st = np.concatenate(
            [local_stats[b][order].T,
             np.broadcast_to(global_stats[b].reshape(1, 1), (1, S))], axis=0)
        m = dict(shared)
        m.update({
            "idx1": idx1, "idx2": idx2, "padm": padm_all[b],
            "statst": np.ascontiguousarray(st).astype(bf),
        })
        in_maps.append(m)
    return in_maps, orders, (slots, ni_ws, v_ws)


last_exec_ns = None
last_results = None


def kernel(**inputs) -> np.ndarray:
    global last_exec_ns, last_results
    in_maps, orders, key = _prep_inputs(**inputs)
    if key not in _cached:
        _cached[key] = _build_program(*key)
    nc = _cached[key]
    trace = bool(int(os.environ.get("KERNEL_TRACE", "0")))
    res = bass_utils.run_bass_kernel_spmd(
        nc, in_maps, core_ids=list(range(B)), trace=trace)
    last_exec_ns = res.exec_time_ns
    last_results = res
    out = np.empty((B, S, H), dtype=np.float32)
    for b in range(B):
        out[b, orders[b]] = res.results[b]["out"]
    return out


if __name__ == "__main__":
    _build_program((33,) * NT, (5504, 5504, 5504, 448), (5500, 5500, 5500, 440))
    print("program builds OK")


# revision 11
# speedup vs baseline: 1.5661x; 1.1035x over previous
"""GAT NodeEncoder kernel for Trainium2 (8 NeuronCores, data-parallel over batch).

Reference computation (per batch element b, per node n):
    src  = E[subgraph[b,n]];  nei_i = E[neighs[b,n,i]]
    s_0  = leaky(src@a1 + src@a2 + a_b); s_i = leaky(src@a1 + nei_i@a2 + a_b) + mask_i*-1e9
    att  = softmax(s); v = sum_i att_i * emb_i
    x = leaky(fc1 @ [v; local_stats; gstat] + b1); out = leaky(fc2 @ x + b2)

Sharding: batch B=8 over 8 cores (1 batch row / core), emb table replicated
(uploaded bf16 -- matches the bf16 in-flight compute precision).

Gather strategy (the SWDGE fixed cost of ~1us/call dominates a naive
128-rows-per-indirect-DMA approach):
  Phase 1: per core, the ~21k UNIQUE referenced table rows are gathered with
  4 dma_gather ucode calls (one per 32768-row window -- dma_gather indices
  are int16) into SBUF, then flushed per-window to a DRAM scratch laid out
  so that scratch rows are addressable by an int16 rank.
  Phase 2: per 128-node tile, ONE dma_gather from the scratch fetches all
  (node, slot) rows positionally (slot-major) -- ~3000 rows per call at
  994ns + 0.34ns/row of gpsimd time.

Compute per tile (slots uniform per tile via degree-sorting, masked
neighbors dropped exactly):
  scores   = reduce_X(g * a2_bcast) on DVE; s = Lrelu(w + (u+ab)) on ACT
  softmax  = max/exp+accum/recip; att = Copy(e, scale=1/z) on ACT
  weighted = g * att (stride-0-inner broadcast) + strided reduce on DVE
  head     = PE transpose via identity, fc1/fc2 on PE, Lrelu+bias on ACT
Output rows are stored directly (HWDGE) in sorted order; host unsorts.
"""

import os
from contextlib import ExitStack

import numpy as np
import ml_dtypes

import concourse.bass as bass
import concourse.bacc as bacc
import concourse.tile as tile
from concourse import mybir
from concourse import bass_utils
from concourse import library_config

B, S, N, H, NLS = 8, 1024, 32, 128, 4
NUM_NODES = 100001
TILE = 128
NT = S // TILE
WIN = 32768
NWIN = 4
F32 = mybir.dt.float32
BF16 = mybir.dt.bfloat16
I32 = mybir.dt.int32
I16 = mybir.dt.int16
AF = mybir.ActivationFunctionType
ALU = mybir.AluOpType

_cached = {}


def _rup(x, m):
    return (x + m - 1) // m * m


def _build_program(slots, ni_ws, v_ws):
    """slots: per-tile slot counts; ni_ws/v_ws: per-window phase-1 static
    num_idxs (mult of 16) and valid counts (SPMD-uniform)."""
    nt = len(slots)
    ctot = int(sum(slots))
    offs = np.concatenate([[0], np.cumsum(slots)]).astype(int)
    blocks_w = [_rup(ni, TILE) // TILE for ni in ni_ws]
    blk_off = np.concatenate([[0], np.cumsum(blocks_w)]).astype(int)
    blk_tot = int(blk_off[-1])
    srows = TILE * blk_tot
    n1cols = [ni // 16 for ni in ni_ws]
    c1off = np.concatenate([[0], np.cumsum(n1cols)]).astype(int)
    n2cols = [TILE * int(c) // 16 for c in slots]
    c2off = np.concatenate([[0], np.cumsum(n2cols)]).astype(int)

    nc = bacc.Bacc(target_bir_lowering=False, debug=False, enable_asserts=False,
                   num_swdge_queues=4)

    emb = nc.dram_tensor("emb", [NUM_NODES, H], BF16, kind="ExternalInput")
    idx1 = nc.dram_tensor("idx1", [TILE, int(c1off[-1])], I16, kind="ExternalInput")
    idx2 = nc.dram_tensor("idx2", [TILE, int(c2off[-1])], I16, kind="ExternalInput")
    padm = nc.dram_tensor("padm", [TILE, ctot], F32, kind="ExternalInput")
    statst = nc.dram_tensor("statst", [NLS + 1, S], BF16, kind="ExternalInput")
    a2rep_d = nc.dram_tensor("a2rep", [1, H], BF16, kind="ExternalInput")
    a1rep_d = nc.dram_tensor("a1rep", [1, H], BF16, kind="ExternalInput")
    ab_rep = nc.dram_tensor("ab_rep", [TILE, 1], F32, kind="ExternalInput")
    ident = nc.dram_tensor("ident", [TILE, TILE], BF16, kind="ExternalInput")
    w1t_a = nc.dram_tensor("w1t_a", [H, H], BF16, kind="ExternalInput")
    w1t_b = nc.dram_tensor("w1t_b", [NLS + 1, H], BF16, kind="ExternalInput")
    b1 = nc.dram_tensor("b1", [H, 1], F32, kind="ExternalInput")
    w2t = nc.dram_tensor("w2t", [H, H], BF16, kind="ExternalInput")
    b2row = nc.dram_tensor("b2row", [1, H], BF16, kind="ExternalInput")
    onesc = nc.dram_tensor("onesc", [1, TILE], BF16, kind="ExternalInput")
    out = nc.dram_tensor("out", [S, H], F32, kind="ExternalOutput")

    cmax = int(max(slots))

    with tile.TileContext(nc) as tc, ExitStack() as ctx:
        dpool = ctx.enter_context(tc.tile_pool(name="dram", bufs=1, space="DRAM"))
        const = ctx.enter_context(tc.tile_pool(name="const", bufs=1))
        gpool = ctx.enter_context(tc.tile_pool(name="gpool", bufs=3))
        spool = ctx.enter_context(tc.tile_pool(name="spool", bufs=3))
        small = ctx.enter_context(tc.tile_pool(name="small", bufs=6))
        opool = ctx.enter_context(tc.tile_pool(name="opool", bufs=2))
        psum = ctx.enter_context(tc.tile_pool(name="psum", bufs=2, space="PSUM"))

        scratch = dpool.tile([srows, H], BF16)

        # ---- constants ----
        c_idx1 = const.tile([TILE, int(c1off[-1])], I16)
        nc.sync.dma_start(out=c_idx1[:], in_=idx1[:, :])
        c_idx2_0 = const.tile([TILE, int(c2off[-1])], I16)
        nc.sync.dma_start(out=c_idx2_0[:], in_=idx2[:, :])
        c_padm0 = const.tile([TILE, ctot], F32)
        nc.sync.dma_start(out=c_padm0[:], in_=padm[:, :])
        c_stats = const.tile([NLS + 1, S], BF16)
        nc.sync.dma_start(out=c_stats[:], in_=statst[:, :])
        c_ab = const.tile([TILE, 1], F32)
        nc.sync.dma_start(out=c_ab[:], in_=ab_rep[:, :])
        c_id = const.tile([TILE, TILE], BF16)
        nc.sync.dma_start(out=c_id[:], in_=ident[:, :])
        c_w1a = const.tile([H, H], BF16)
        nc.sync.dma_start(out=c_w1a[:], in_=w1t_a[:, :])
        c_w1b = const.tile([NLS + 1, H], BF16)
        nc.sync.dma_start(out=c_w1b[:], in_=w1t_b[:, :])
        c_b1 = const.tile([H, 1], F32)
        nc.sync.dma_start(out=c_b1[:], in_=b1[:, :])
        c_w2 = const.tile([H, H], BF16)
        nc.sync.dma_start(out=c_w2[:], in_=w2t[:, :])
        c_b2 = const.tile([1, H], BF16)
        nc.sync.dma_start(out=c_b2[:], in_=b2row[:, :])
        c_ones = const.tile([1, TILE], BF16)
        nc.sync.dma_start(out=c_ones[:], in_=onesc[:, :])
        # a1/a2 replicated to 128 partitions (DMA broadcast)
        c_a2r0 = const.tile([TILE, H], BF16)
        nc.gpsimd.dma_start(out=c_a2r0[:], in_=bass.AP(
            tensor=a2rep_d.ap().tensor, offset=0, ap=[[0, TILE], [1, H]]))
        c_a1r0 = const.tile([TILE, H], BF16)
        nc.gpsimd.dma_start(out=c_a1r0[:], in_=bass.AP(
            tensor=a1rep_d.ap().tensor, offset=0, ap=[[0, TILE], [1, H]]))

        nc.gpsimd.load_library(library_config.mlp)

        # ---- phase 1: gather unique rows per 32768-row window; each window
        # split into <=4 chunks round-robined over the 4 SWDGE queues ----
        g1 = const.tile([TILE, blk_tot * H], BF16)
        qrr = 0
        for w in range(NWIN):
            if ni_ws[w] == 0:
                continue
            span = min(WIN, NUM_NODES - w * WIN)
            src_ap = bass.AP(tensor=emb.ap().tensor, offset=w * WIN * H,
                             ap=[[H, span], [1, H]])
            niw, vw = int(ni_ws[w]), int(v_ws[w])
            if niw >= 2048:
                q = _rup((niw + 3) // 4, TILE)
                chunks = [(p0, min(q, niw - p0)) for p0 in range(0, niw, q)]
            else:
                chunks = [(0, niw)]
            for (p0, ln) in chunks:
                vc = max(0, min(ln, vw - p0))
                b0 = int(blk_off[w]) + p0 // TILE
                nblk = _rup(ln, TILE) // TILE
                nc.gpsimd.dma_gather(
                    g1[:, b0 * H:(b0 + nblk) * H].rearrange(
                        "p (b h) -> p b h", b=nblk),
                    src_ap,
                    c_idx1[:, int(c1off[w]) + p0 // 16:int(c1off[w]) + (p0 + ln) // 16],
                    ln,
                    vc,
                    H,
                    single_packet=False,
                    queue_num=qrr % 4,
                )
                qrr += 1
            # flush window w to scratch rows p*blk_tot + blk_off[w] + u
            nc.sync.dma_start(
                out=bass.AP(
                    tensor=scratch[:].tensor,
                    offset=scratch[:].offset + int(blk_off[w]) * H,
                    ap=[[blk_tot * H, TILE], [H, blocks_w[w]], [1, H]]),
                in_=g1[:, int(blk_off[w]) * H:int(blk_off[w + 1]) * H])

        # ---- one-time fences: absorb const-DMA sems onto consuming engines ----
        c_idx2 = const.tile([TILE, int(c2off[-1])], I16)
        nc.vector.tensor_copy(out=c_idx2[:], in_=c_idx2_0[:])
        c_a2r = const.tile([TILE, H], BF16)
        nc.vector.tensor_copy(out=c_a2r[:], in_=c_a2r0[:])
        c_a1r = const.tile([TILE, H], BF16)
        nc.vector.tensor_copy(out=c_a1r[:], in_=c_a1r0[:])
        c_padm = const.tile([TILE, ctot], F32)
        nc.vector.tensor_copy(out=c_padm[:], in_=c_padm0[:])
        c_ab2 = const.tile([TILE, 1], F32)
        nc.vector.tensor_copy(out=c_ab2[:], in_=c_ab[:])
        dpsum = psum.tile([TILE, TILE], F32, tag="dfence")
        nc.tensor.matmul(out=dpsum[:], lhsT=c_id[:], rhs=c_w1a[:], start=True, stop=True)
        nc.tensor.matmul(out=dpsum[:], lhsT=c_w2[:], rhs=c_id[:], start=True, stop=True)
        nc.tensor.matmul(
            out=dpsum[:], lhsT=c_w1b[:], rhs=c_stats[:, 0:TILE], start=True, stop=True)
        nc.tensor.matmul(out=dpsum[:], lhsT=c_ones[:], rhs=c_b2[:], start=True, stop=True)
        dact = const.tile([TILE, 1], F32)
        nc.scalar.activation(out=dact[:], in_=c_ab2[:], func=AF.Identity, bias=c_b1[:, 0:1])

        # ---- phase 2: per-tile positional gather + compute ----
        for t in range(nt):
            ct = int(slots[t])
            o0 = int(offs[t])
            g = gpool.tile([TILE, cmax * H], BF16, tag="g")
            nc.gpsimd.dma_gather(
                g[:, :ct * H].rearrange("p (i h) -> p i h", i=ct),
                scratch[:],
                c_idx2[:, int(c2off[t]):int(c2off[t + 1])],
                TILE * ct,
                TILE * ct,
                H,
                single_packet=False,
                queue_num=t % 4,
            )

            # ---- scores: w[:, i] = g_i . a2  (broadcast mul + seg reduce) ----
            t1 = spool.tile([TILE, cmax * H], BF16, tag="t1")
            a2b = bass.AP(tensor=c_a2r[:].tensor, offset=c_a2r[:].offset,
                          ap=[c_a2r[:].ap[0], [0, ct], [1, H]])
            nc.vector.tensor_tensor(
                out=t1[:, :ct * H].rearrange("p (i h) -> p i h", i=ct),
                in0=g[:, :ct * H].rearrange("p (i h) -> p i h", i=ct),
                in1=a2b, op=ALU.mult)
            w = small.tile([TILE, cmax], F32, tag="w")
            nc.vector.reduce_sum(
                out=w[:, :ct],
                in_=t1[:, :ct * H].rearrange("p (i h) -> p i h", i=ct),
                axis=mybir.AxisListType.X)
            # u = src . a1 (slot 0), then u' = u + a_b
            t2 = small.tile([TILE, H], BF16, tag="t2")
            nc.vector.tensor_tensor(out=t2[:], in0=g[:, :H], in1=c_a1r[:], op=ALU.mult)
            u = small.tile([TILE, 1], F32, tag="u")
            nc.vector.reduce_sum(
                out=u[:], in_=t2[:].rearrange("p (i h) -> p i h", i=1),
                axis=mybir.AxisListType.X)
            up = small.tile([TILE, 1], F32, tag="up")
            nc.vector.tensor_scalar(
                out=up[:], in0=u[:], scalar1=c_ab2[:, 0:1], scalar2=None,
                op0=ALU.add)
            # s = leaky(w + u'), then -1e9 on pad slots
            s0 = small.tile([TILE, cmax], F32, tag="s0")
            nc.scalar.activation(
                out=s0[:, :ct], in_=w[:, :ct], func=AF.Identity, bias=up[:, 0:1])
            s = small.tile([TILE, cmax], F32, tag="s")
            nc.vector.scalar_tensor_tensor(
                out=s[:, :ct], in0=s0[:, :ct], scalar=0.2, in1=s0[:, :ct],
                op0=ALU.mult, op1=ALU.max)
            nc.vector.scalar_tensor_tensor(
                out=s[:, :ct], in0=c_padm[:, o0:o0 + ct], scalar=-1e9,
                in1=s[:, :ct], op0=ALU.mult, op1=ALU.add)
            # softmax
            negm = small.tile([TILE, 1], F32, tag="negm")
            nc.vector.tensor_reduce(
                out=negm[:], in_=s[:, :ct], axis=mybir.AxisListType.X, op=ALU.max,
                negate=True)
            e = small.tile([TILE, cmax], F32, tag="e")
            zsum = small.tile([TILE, 1], F32, tag="zsum")
            nc.scalar.activation(
                out=e[:, :ct], in_=s[:, :ct], func=AF.Exp, bias=negm[:, 0:1],
                accum_out=zsum[:])
            r = small.tile([TILE, 1], F32, tag="r")
            nc.vector.reciprocal(out=r[:], in_=zsum[:])
            att = small.tile([TILE, cmax], F32, tag="att")
            nc.scalar.activation(
                out=att[:, :ct], in_=e[:, :ct], func=AF.Copy, scale=r[:, 0:1])

            # ---- weighted sum: gs = g * att (stride-0-inner bcast), vsum ----
            gs = spool.tile([TILE, cmax * H], BF16, tag="gs")
            attb = bass.AP(tensor=att[:].tensor, offset=att[:].offset,
                           ap=[att[:].ap[0], [1, ct], [0, H]])
            nc.vector.tensor_tensor(
                out=gs[:, :ct * H].rearrange("p (i h) -> p i h", i=ct),
                in0=g[:, :ct * H].rearrange("p (i h) -> p i h", i=ct),
                in1=attb, op=ALU.mult)
            # halving add-tree over slots (gs pad slots are exactly 0);
            # final level emitted in f32
            k = ct
            while k > 2:
                half = k // 2
                nc.vector.tensor_tensor(
                    out=gs[:, :half * H], in0=gs[:, :half * H],
                    in1=gs[:, half * H:2 * half * H], op=ALU.add)
                if k - 2 * half:
                    nc.vector.tensor_tensor(
                        out=gs[:, (half - 1) * H:half * H],
                        in0=gs[:, (half - 1) * H:half * H],
                        in1=gs[:, (k - 1) * H:k * H], op=ALU.add)
                k = half
            v = small.tile([TILE, H], F32, tag="v")
            nc.vector.tensor_tensor(
                out=v[:], in0=gs[:, :H], in1=gs[:, H:2 * H], op=ALU.add)
            vb = small.tile([TILE, H], BF16, tag="vb")
            nc.scalar.activation(out=vb[:], in_=v[:], func=AF.Copy)

            # ---- transpose v via PE identity ----
            vps = psum.tile([H, TILE], F32, tag="vps")
            nc.tensor.matmul(out=vps[:], lhsT=vb[:], rhs=c_id[:], start=True, stop=True)
            vt = small.tile([H, TILE], BF16, tag="vt")
            nc.scalar.activation(out=vt[:], in_=vps[:], func=AF.Copy)

            # ---- MLP head ----
            o1p = psum.tile([H, TILE], F32, tag="o1p")
            nc.tensor.matmul(out=o1p[:], lhsT=c_w1a[:], rhs=vt[:], start=True, stop=False)
            nc.tensor.matmul(
                out=o1p[:], lhsT=c_w1b[:], rhs=c_stats[:, t * TILE:(t + 1) * TILE],
                start=False, stop=True)
            o1c = small.tile([H, TILE], BF16, tag="o1c")
            nc.scalar.activation(out=o1c[:], in_=o1p[:], func=AF.Identity, bias=c_b1[:, 0:1])
            o1 = small.tile([H, TILE], BF16, tag="o1")
            nc.vector.scalar_tensor_tensor(
                out=o1[:], in0=o1c[:], scalar=0.2, in1=o1c[:], op0=ALU.mult, op1=ALU.max)
            o2p = psum.tile([TILE, H], F32, tag="o2p")
            nc.tensor.matmul(out=o2p[:], lhsT=o1[:], rhs=c_w2[:], start=True, stop=False)
            nc.tensor.matmul(out=o2p[:], lhsT=c_ones[:], rhs=c_b2[:], start=False, stop=True)
            otc = small.tile([TILE, H], F32, tag="otc")
            nc.scalar.activation(out=otc[:], in_=o2p[:], func=AF.Copy)
            ot = opool.tile([TILE, H], F32, tag="ot")
            nc.vector.scalar_tensor_tensor(
                out=ot[:], in0=otc[:], scalar=0.2, in1=otc[:], op0=ALU.mult, op1=ALU.max)
            # direct store in sorted order (host unsorts)
            nc.sync.dma_start(
                out=bass.AP(tensor=out.ap().tensor, offset=t * TILE * H,
                            ap=[[H, TILE], [1, H]]),
                in_=ot[:])

    nc.finalize()
    return nc


def _prep_inputs(subgraph, neighs, mask, local_stats, global_stats,
                 emb_table, a_w, a_b, fc1_w, fc1_b, fc2_w, fc2_b):
    """Host-side layout/sharding prep.

    Returns (in_maps, orders, key) where key = (slots, ni_ws, v_ws)."""
    bf = ml_dtypes.bfloat16
    a1 = a_w[0, :H]
    a2 = a_w[0, H:]
    shared = {
        "emb": np.ascontiguousarray(emb_table).astype(bf),
        "a2rep": a2.reshape(1, H).astype(bf),
        "a1rep": a1.reshape(1, H).astype(bf),
        "ab_rep": np.broadcast_to(a_b.astype(np.float32), (TILE, 1)).copy(),
        "ident": np.eye(TILE, dtype=np.float32).astype(bf),
        "w1t_a": np.ascontiguousarray(fc1_w[:, :H].T).astype(bf),
        "w1t_b": np.ascontiguousarray(fc1_w[:, H:].T).astype(bf),
        "b1": fc1_b.reshape(H, 1).astype(np.float32),
        "w2t": np.ascontiguousarray(fc2_w.T).astype(bf),
        "b2row": fc2_b.reshape(1, H).astype(bf),
        "onesc": np.ones((1, TILE), dtype=np.float32).astype(bf),
    }
    keep = mask[:, :, :, 0] < 0.5          # [B,S,N] True = neighbor survives
    counts = 1 + keep.sum(axis=2)          # [B,S]
    orders = np.argsort(-counts, axis=1, kind="stable")

    slots = []
    for t in range(NT):
        c = 0
        for b in range(B):
            c = max(c, int(counts[b, orders[b, t * TILE]]))
        slots.append(c)
    slots = tuple(slots)
    offs = np.concatenate([[0], np.cumsum(slots)]).astype(int)
    ctot = int(offs[-1])

    # absolute slot ids per core (-1 = pad)
    idx_abs_all, padm_all, uniq_all = [], [], []
    for b in range(B):
        order = orders[b]
        idx_abs = np.full((TILE, ctot), -1, dtype=np.int64)
        padm = np.zeros((TILE, ctot), dtype=np.float32)
        for t in range(NT):
            ct = slots[t]
            o0 = offs[t]
            nodes = order[t * TILE:(t + 1) * TILE]
            idx_abs[:, o0] = subgraph[b, nodes]
            for p in range(TILE):
                n = nodes[p]
                kn = neighs[b, n][keep[b, n]]
                idx_abs[p, o0 + 1:o0 + 1 + len(kn)] = kn
                padm[p, o0 + 1 + len(kn):o0 + ct] = 1.0
        idx_abs_all.append(idx_abs)
        padm_all.append(padm)
        uniq_all.append(np.unique(idx_abs[idx_abs >= 0]))

    # per-window unique counts; SPMD-uniform valid counts
    wstarts = []
    for b in range(B):
        ws = np.searchsorted(uniq_all[b], np.arange(NWIN + 1) * WIN)
        wstarts.append(ws)
    v_ws = tuple(int(max(wstarts[b][w + 1] - wstarts[b][w] for b in range(B)))
                 for w in range(NWIN))
    ni_ws = tuple(_rup(v, 16) for v in v_ws)
    blocks_w = [_rup(ni, TILE) // TILE for ni in ni_ws]
    blk_off = np.concatenate([[0], np.cumsum(blocks_w)]).astype(int)
    blk_tot = int(blk_off[-1])
    assert TILE * blk_tot < 32768, f"scratch rows {TILE * blk_tot} exceed int16"

    def wrap16(a):
        return np.ascontiguousarray(a.reshape(-1, 16).T)

    in_maps = []
    for b in range(B):
        uniq = uniq_all[b]
        ws = wstarts[b]
        # phase-1 window index lists
        idx1_parts = []
        row_of_rank = np.empty(len(uniq), dtype=np.int64)
        for w in range(NWIN):
            if ni_ws[w] == 0:
                continue
            rel = uniq[ws[w]:ws[w + 1]] - w * WIN
            nwb = len(rel)
            a = np.full(ni_ws[w], -1, dtype=np.int16)
            a[:nwb] = rel.astype(np.int16)
            a[nwb:v_ws[w]] = 0                     # top-up (dup row) for SPMD
            idx1_parts.append(wrap16(a))
            j = np.arange(nwb)
            row_of_rank[ws[w]:ws[w + 1]] = (j % TILE) * blk_tot + blk_off[w] + j // TILE
        idx1 = np.tile(np.concatenate(idx1_parts, axis=1), (8, 1))

        # phase-2 positional rank lists (slot-major per tile)
        idx_abs = idx_abs_all[b]
        ranks = np.searchsorted(uniq, np.clip(idx_abs, 0, None))
        row2 = row_of_rank[ranks]
        row2[idx_abs < 0] = row_of_rank[0]         # pads -> a known-valid row
        assert row2.max() < 32768
        idx2_parts = []
        for t in range(NT):
            ct = slots[t]
            o0 = offs[t]
            arr = np.ascontiguousarray(row2[:, o0:o0 + ct].T).ravel()  # j = i*128+p
            idx2_parts.append(wrap16(arr.astype(np.int16)))
        idx2 = np.tile(np.concatenate(idx2_parts, axis=1), (8, 1))

        order = orders[b]
        st = np.concatenate(
            [local_stats[b][order].T,
             np.broadcast_to(global_stats[b].reshape(1, 1), (1, S))], axis=0)
        m = dict(shared)
        m.update({
            "idx1": idx1, "idx2": idx2, "padm": padm_all[b],
            "statst": np.ascontiguousarray(st).astype(bf),
        })
        in_maps.append(m)
    return in_maps, orders, (slots, ni_ws, v_ws)


last_exec_ns = None
last_results = None


def kernel(**inputs) -> np.ndarray:
    global last_exec_ns, last_results
    in_maps, orders, key = _prep_inputs(**inputs)
    if key not in _cached:
        _cached[key] = _build_program(*key)
    nc = _cached[key]
    trace = bool(int(os.environ.get("KERNEL_TRACE", "0")))
    res = bass_utils.run_bass_kernel_spmd(
        nc, in_maps, core_ids=list(range(B)), trace=trace)
    last_exec_ns = res.exec_time_ns
    last_results = res
    out = np.empty((B, S, H), dtype=np.float32)
    for b in range(B):
        out[b, orders[b]] = res.results[b]["out"]
    return out


if __name__ == "__main__":
    _build_program((33,) * NT, (5504, 5504, 5504, 448), (5500, 5500, 5500, 440))
    print("program builds OK")


# revision 14
# speedup vs baseline: 2.0630x; 1.3173x over previous
"""GAT NodeEncoder kernel for Trainium2 (8 NeuronCores, data-parallel over batch).

Reference computation (per batch element b, per node n):
    src  = E[subgraph[b,n]];  nei_i = E[neighs[b,n,i]]
    s_0  = leaky(src@a1 + src@a2 + a_b); s_i = leaky(src@a1 + nei_i@a2 + a_b) + mask_i*-1e9
    att  = softmax(s); v = sum_i att_i * emb_i
    x = leaky(fc1 @ [v; local_stats; gstat] + b1); out = leaky(fc2 @ x + b2)

Sharding: batch B=8 over 8 cores (1 batch row / core), emb table replicated
(uploaded bf16 -- matches the bf16 in-flight compute precision).

Gather strategy (the SWDGE fixed cost of ~1us/call dominates a naive
128-rows-per-indirect-DMA approach):
  Phase 1: per core, the ~21k UNIQUE referenced table rows are gathered with
  4 dma_gather ucode calls (one per 32768-row window -- dma_gather indices
  are int16) into SBUF, then flushed per-window to a DRAM scratch laid out
  so that scratch rows are addressable by an int16 rank.
  Phase 2: per 128-node tile, ONE dma_gather from the scratch fetches all
  (node, slot) rows positionally (slot-major) -- ~3000 rows per call at
  994ns + 0.34ns/row of gpsimd time.

Compute per tile (slots uniform per tile via degree-sorting, masked
neighbors dropped exactly):
  scores   = reduce_X(g * a2_bcast) on DVE; s = Lrelu(w + (u+ab)) on ACT
  softmax  = max/exp+accum/recip; att = Copy(e, scale=1/z) on ACT
  weighted = g * att (stride-0-inner broadcast) + strided reduce on DVE
  head     = PE transpose via identity, fc1/fc2 on PE, Lrelu+bias on ACT
Output rows are stored directly (HWDGE) in sorted order; host unsorts.
"""

import os
from contextlib import ExitStack

import numpy as np
import ml_dtypes

import concourse.bass as bass
import concourse.bacc as bacc
import concourse.tile as tile
from concourse import mybir
from concourse import bass_utils
from concourse import library_config

B, S, N, H, NLS = 8, 1024, 32, 128, 4
NUM_NODES = 100001
TILE = 128
NT = S // TILE
WIN = 32768
NWIN = 4
F32 = mybir.dt.float32
BF16 = mybir.dt.bfloat16
I32 = mybir.dt.int32
I16 = mybir.dt.int16
AF = mybir.ActivationFunctionType
ALU = mybir.AluOpType

_cached = {}


def _rup(x, m):
    return (x + m - 1) // m * m


def _build_program(slots, ni_ws, v_ws):
    """slots: per-tile slot counts; ni_ws/v_ws: per-window phase-1 static
    num_idxs (mult of 16) and valid counts (SPMD-uniform)."""
    nt = len(slots)
    ctot = int(sum(slots))
    offs = np.concatenate([[0], np.cumsum(slots)]).astype(int)
    blocks_w = [_rup(ni, TILE) // TILE for ni in ni_ws]
    blk_off = np.concatenate([[0], np.cumsum(blocks_w)]).astype(int)
    blk_tot = int(blk_off[-1])
    srows = TILE * blk_tot
    n1cols = [ni // 16 for ni in ni_ws]
    c1off = np.concatenate([[0], np.cumsum(n1cols)]).astype(int)
    n2cols = [TILE * int(c) // 16 for c in slots]
    c2off = np.concatenate([[0], np.cumsum(n2cols)]).astype(int)

    nc = bacc.Bacc(target_bir_lowering=False, debug=False, enable_asserts=False,
                   num_swdge_queues=4)

    emb = nc.dram_tensor("emb", [NUM_NODES, H], BF16, kind="ExternalInput")
    idx1 = nc.dram_tensor("idx1", [TILE, int(c1off[-1])], I16, kind="ExternalInput")
    idx2 = nc.dram_tensor("idx2", [TILE, int(c2off[-1])], I16, kind="ExternalInput")
    padm = nc.dram_tensor("padm", [TILE, ctot], F32, kind="ExternalInput")
    statst = nc.dram_tensor("statst", [NLS + 1, S], BF16, kind="ExternalInput")
    a2rep_d = nc.dram_tensor("a2rep", [1, H], BF16, kind="ExternalInput")
    a1rep_d = nc.dram_tensor("a1rep", [1, H], BF16, kind="ExternalInput")
    ab_rep = nc.dram_tensor("ab_rep", [TILE, 1], F32, kind="ExternalInput")
    ident = nc.dram_tensor("ident", [TILE, TILE], BF16, kind="ExternalInput")
    w1t_a = nc.dram_tensor("w1t_a", [H, H], BF16, kind="ExternalInput")
    w1t_b = nc.dram_tensor("w1t_b", [NLS + 1, H], BF16, kind="ExternalInput")
    b1 = nc.dram_tensor("b1", [H, 1], F32, kind="ExternalInput")
    w2t = nc.dram_tensor("w2t", [H, H], BF16, kind="ExternalInput")
    b2row = nc.dram_tensor("b2row", [1, H], BF16, kind="ExternalInput")
    onesc = nc.dram_tensor("onesc", [1, TILE], BF16, kind="ExternalInput")
    out = nc.dram_tensor("out", [S, H], F32, kind="ExternalOutput")

    cmax = int(max(slots))

    with tile.TileContext(nc) as tc, ExitStack() as ctx:
        dpool = ctx.enter_context(tc.tile_pool(name="dram", bufs=1, space="DRAM"))
        const = ctx.enter_context(tc.tile_pool(name="const", bufs=1))
        psum = ctx.enter_context(tc.tile_pool(name="psum", bufs=2, space="PSUM"))

        scratch = dpool.tile([srows, H], BF16)

        # ---- constants ----
        c_idx2_0 = const.tile([TILE, int(c2off[-1])], I16)
        nc.sync.dma_start(out=c_idx2_0[:], in_=idx2[:, :])
        c_padm0 = const.tile([TILE, ctot], F32)
        nc.sync.dma_start(out=c_padm0[:], in_=padm[:, :])
        c_stats = const.tile([NLS + 1, S], BF16)
        nc.sync.dma_start(out=c_stats[:], in_=statst[:, :])
        c_ab = const.tile([TILE, 1], F32)
        nc.sync.dma_start(out=c_ab[:], in_=ab_rep[:, :])
        c_id = const.tile([TILE, TILE], BF16)
        nc.sync.dma_start(out=c_id[:], in_=ident[:, :])
        c_w1a = const.tile([H, H], BF16)
        nc.sync.dma_start(out=c_w1a[:], in_=w1t_a[:, :])
        c_w1b = const.tile([NLS + 1, H], BF16)
        nc.sync.dma_start(out=c_w1b[:], in_=w1t_b[:, :])
        c_b1 = const.tile([H, 1], F32)
        nc.sync.dma_start(out=c_b1[:], in_=b1[:, :])
        c_w2 = const.tile([H, H], BF16)
        nc.sync.dma_start(out=c_w2[:], in_=w2t[:, :])
        c_b2 = const.tile([1, H], BF16)
        nc.sync.dma_start(out=c_b2[:], in_=b2row[:, :])
        c_ones = const.tile([1, TILE], BF16)
        nc.sync.dma_start(out=c_ones[:], in_=onesc[:, :])
        # a1/a2 replicated to 128 partitions (DMA broadcast)
        c_a2r0 = const.tile([TILE, H], BF16)
        nc.gpsimd.dma_start(out=c_a2r0[:], in_=bass.AP(
            tensor=a2rep_d.ap().tensor, offset=0, ap=[[0, TILE], [1, H]]))
        c_a1r0 = const.tile([TILE, H], BF16)
        nc.gpsimd.dma_start(out=c_a1r0[:], in_=bass.AP(
            tensor=a1rep_d.ap().tensor, offset=0, ap=[[0, TILE], [1, H]]))

        nc.gpsimd.load_library(library_config.mlp)

        # ---- phase 1: gather unique rows per 32768-row window; each window
        # split into <=4 chunks round-robined over the 4 SWDGE queues.
        # Runs in its own pool so the staging tile frees before phase 2. ----
        with tc.tile_pool(name="p1", bufs=1) as p1pool:
            c_idx1 = p1pool.tile([TILE, int(c1off[-1])], I16)
            nc.sync.dma_start(out=c_idx1[:], in_=idx1[:, :])
            g1 = p1pool.tile([TILE, blk_tot * H], BF16)
            qrr = 0
            for w in range(NWIN):
                if ni_ws[w] == 0:
                    continue
                span = min(WIN, NUM_NODES - w * WIN)
                src_ap = bass.AP(tensor=emb.ap().tensor, offset=w * WIN * H,
                                 ap=[[H, span], [1, H]])
                niw, vw = int(ni_ws[w]), int(v_ws[w])
                if niw >= 2048:
                    q = _rup((niw + 3) // 4, TILE)
                    chunks = [(p0, min(q, niw - p0)) for p0 in range(0, niw, q)]
                else:
                    chunks = [(0, niw)]
                for (p0, ln) in chunks:
                    vc = max(0, min(ln, vw - p0))
                    b0 = int(blk_off[w]) + p0 // TILE
                    nblk = _rup(ln, TILE) // TILE
                    nc.gpsimd.dma_gather(
                        g1[:, b0 * H:(b0 + nblk) * H].rearrange(
                            "p (b h) -> p b h", b=nblk),
                        src_ap,
                        c_idx1[:, int(c1off[w]) + p0 // 16:
                               int(c1off[w]) + (p0 + ln) // 16],
                        ln,
                        vc,
                        H,
                        single_packet=False,
                        queue_num=qrr % 4,
                    )
                    qrr += 1
                # flush window w to scratch rows p*blk_tot + blk_off[w] + u
                nc.sync.dma_start(
                    out=bass.AP(
                        tensor=scratch[:].tensor,
                        offset=scratch[:].offset + int(blk_off[w]) * H,
                        ap=[[blk_tot * H, TILE], [H, blocks_w[w]], [1, H]]),
                    in_=g1[:, int(blk_off[w]) * H:int(blk_off[w + 1]) * H])

        gpool = ctx.enter_context(tc.tile_pool(name="gpool", bufs=1))
        spool = ctx.enter_context(tc.tile_pool(name="spool", bufs=3))
        small = ctx.enter_context(tc.tile_pool(name="small", bufs=6))
        opool = ctx.enter_context(tc.tile_pool(name="opool", bufs=2))

        # ---- one-time fences: absorb const-DMA sems onto consuming engines ----
        c_idx2 = const.tile([TILE, int(c2off[-1])], I16)
        nc.vector.tensor_copy(out=c_idx2[:], in_=c_idx2_0[:])
        c_a2r = const.tile([TILE, H], BF16)
        nc.vector.tensor_copy(out=c_a2r[:], in_=c_a2r0[:])
        c_a1r = const.tile([TILE, H], BF16)
        nc.vector.tensor_copy(out=c_a1r[:], in_=c_a1r0[:])
        c_padm = const.tile([TILE, ctot], F32)
        nc.vector.tensor_copy(out=c_padm[:], in_=c_padm0[:])
        c_ab2 = const.tile([TILE, 1], F32)
        nc.vector.tensor_copy(out=c_ab2[:], in_=c_ab[:])
        dpsum = psum.tile([TILE, TILE], F32, tag="dfence")
        nc.tensor.matmul(out=dpsum[:], lhsT=c_id[:], rhs=c_w1a[:], start=True, stop=True)
        nc.tensor.matmul(out=dpsum[:], lhsT=c_w2[:], rhs=c_id[:], start=True, stop=True)
        nc.tensor.matmul(
            out=dpsum[:], lhsT=c_w1b[:], rhs=c_stats[:, 0:TILE], start=True, stop=True)
        nc.tensor.matmul(out=dpsum[:], lhsT=c_ones[:], rhs=c_b2[:], start=True, stop=True)
        dact = const.tile([TILE, 1], F32)
        nc.scalar.activation(out=dact[:], in_=c_ab2[:], func=AF.Identity, bias=c_b1[:, 0:1])

        # ---- phase 2: ALL positional gathers issued back-to-back (4-queue
        # parallel), then the per-tile compute loop ----
        gtiles = []
        for t in range(nt):
            ct = int(slots[t])
            g = gpool.tile([TILE, ct * H], BF16, tag=f"g{t}")
            nc.gpsimd.dma_gather(
                g[:].rearrange("p (i h) -> p i h", i=ct),
                scratch[:],
                c_idx2[:, int(c2off[t]):int(c2off[t + 1])],
                TILE * ct,
                TILE * ct,
                H,
                single_packet=False,
                queue_num=t % 4,
            )
            gtiles.append(g)

        for t in range(nt):
            ct = int(slots[t])
            o0 = int(offs[t])
            g = gtiles[t]

            # ---- scores: w[:, i] = g_i . a2  (broadcast mul + seg reduce) ----
            t1 = spool.tile([TILE, cmax * H], BF16, tag="t1")
            a2b = bass.AP(tensor=c_a2r[:].tensor, offset=c_a2r[:].offset,
                          ap=[c_a2r[:].ap[0], [0, ct], [1, H]])
            nc.vector.tensor_tensor(
                out=t1[:, :ct * H].rearrange("p (i h) -> p i h", i=ct),
                in0=g[:, :ct * H].rearrange("p (i h) -> p i h", i=ct),
                in1=a2b, op=ALU.mult)
            w = small.tile([TILE, cmax], F32, tag="w")
            nc.vector.reduce_sum(
                out=w[:, :ct],
                in_=t1[:, :ct * H].rearrange("p (i h) -> p i h", i=ct),
                axis=mybir.AxisListType.X)
            # u = src . a1 (slot 0), then u' = u + a_b
            t2 = small.tile([TILE, H], BF16, tag="t2")
            nc.vector.tensor_tensor(out=t2[:], in0=g[:, :H], in1=c_a1r[:], op=ALU.mult)
            u = small.tile([TILE, 1], F32, tag="u")
            nc.vector.reduce_sum(
                out=u[:], in_=t2[:].rearrange("p (i h) -> p i h", i=1),
                axis=mybir.AxisListType.X)
            up = small.tile([TILE, 1], F32, tag="up")
            nc.vector.tensor_scalar(
                out=up[:], in0=u[:], scalar1=c_ab2[:, 0:1], scalar2=None,
                op0=ALU.add)
            # s = leaky(w + u'), then -1e9 on pad slots
            s0 = small.tile([TILE, cmax], F32, tag="s0")
            nc.scalar.activation(
                out=s0[:, :ct], in_=w[:, :ct], func=AF.Identity, bias=up[:, 0:1])
            s = small.tile([TILE, cmax], F32, tag="s")
            nc.vector.scalar_tensor_tensor(
                out=s[:, :ct], in0=s0[:, :ct], scalar=0.2, in1=s0[:, :ct],
                op0=ALU.mult, op1=ALU.max)
            nc.vector.scalar_tensor_tensor(
                out=s[:, :ct], in0=c_padm[:, o0:o0 + ct], scalar=-1e9,
                in1=s[:, :ct], op0=ALU.mult, op1=ALU.add)
            # softmax
            negm = small.tile([TILE, 1], F32, tag="negm")
            nc.vector.tensor_reduce(
                out=negm[:], in_=s[:, :ct], axis=mybir.AxisListType.X, op=ALU.max,
                negate=True)
            e = small.tile([TILE, cmax], F32, tag="e")
            zsum = small.tile([TILE, 1], F32, tag="zsum")
            nc.scalar.activation(
                out=e[:, :ct], in_=s[:, :ct], func=AF.Exp, bias=negm[:, 0:1],
                accum_out=zsum[:])
            r = small.tile([TILE, 1], F32, tag="r")
            nc.vector.reciprocal(out=r[:], in_=zsum[:])
            att = small.tile([TILE, cmax], F32, tag="att")
            nc.scalar.activation(
                out=att[:, :ct], in_=e[:, :ct], func=AF.Copy, scale=r[:, 0:1])

            # ---- weighted sum: gs = g * att (stride-0-inner bcast), vsum ----
            gs = spool.tile([TILE, cmax * H], BF16, tag="gs")
            attb = bass.AP(tensor=att[:].tensor, offset=att[:].offset,
                           ap=[att[:].ap[0], [1, ct], [0, H]])
            nc.vector.tensor_tensor(
                out=gs[:, :ct * H].rearrange("p (i h) -> p i h", i=ct),
                in0=g[:, :ct * H].rearrange("p (i h) -> p i h", i=ct),
                in1=attb, op=ALU.mult)
            # halving add-tree over slots (gs pad slots are exactly 0);
            # final level emitted in f32
            k = ct
            while k > 2:
                half = k // 2
                nc.vector.tensor_tensor(
                    out=gs[:, :half * H], in0=gs[:, :half * H],
                    in1=gs[:, half * H:2 * half * H], op=ALU.add)
                if k - 2 * half:
                    nc.vector.tensor_tensor(
                        out=gs[:, (half - 1) * H:half * H],
                        in0=gs[:, (half - 1) * H:half * H],
                        in1=gs[:, (k - 1) * H:k * H], op=ALU.add)
                k = half
            v = small.tile([TILE, H], F32, tag="v")
            nc.vector.tensor_tensor(
                out=v[:], in0=gs[:, :H], in1=gs[:, H:2 * H], op=ALU.add)
            vb = small.tile([TILE, H], BF16, tag="vb")
            nc.scalar.activation(out=vb[:], in_=v[:], func=AF.Copy)

            # ---- transpose v via PE identity ----
            vps = psum.tile([H, TILE], F32, tag="vps")
            nc.tensor.matmul(out=vps[:], lhsT=vb[:], rhs=c_id[:], start=True, stop=True)
            vt = small.tile([H, TILE], BF16, tag="vt")
            nc.scalar.activation(out=vt[:], in_=vps[:], func=AF.Copy)

            # ---- MLP head ----
            o1p = psum.tile([H, TILE], F32, tag="o1p")
            nc.tensor.matmul(out=o1p[:], lhsT=c_w1a[:], rhs=vt[:], start=True, stop=False)
            nc.tensor.matmul(
                out=o1p[:], lhsT=c_w1b[:], rhs=c_stats[:, t * TILE:(t + 1) * TILE],
                start=False, stop=True)
            o1c = small.tile([H, TILE], BF16, tag="o1c")
            nc.scalar.activation(out=o1c[:], in_=o1p[:], func=AF.Identity, bias=c_b1[:, 0:1])
            o1 = small.tile([H, TILE], BF16, tag="o1")
            nc.vector.scalar_tensor_tensor(
                out=o1[:], in0=o1c[:], scalar=0.2, in1=o1c[:], op0=ALU.mult, op1=ALU.max)
            o2p = psum.tile([TILE, H], F32, tag="o2p")
            nc.tensor.matmul(out=o2p[:], lhsT=o1[:], rhs=c_w2[:], start=True, stop=False)
            nc.tensor.matmul(out=o2p[:], lhsT=c_ones[:], rhs=c_b2[:], start=False, stop=True)
            otc = small.tile([TILE, H], F32, tag="otc")
            nc.scalar.activation(out=otc[:], in_=o2p[:], func=AF.Copy)
            ot = opool.tile([TILE, H], F32, tag="ot")
            nc.vector.scalar_tensor_tensor(
                out=ot[:], in0=otc[:], scalar=0.2, in1=otc[:], op0=ALU.mult, op1=ALU.max)
            # direct store in sorted order (host unsorts)
            nc.sync.dma_start(
                out=bass.AP(tensor=out.ap().tensor, offset=t * TILE * H,
                            ap=[[H, TILE], [1, H]]),
                in_=ot[:])

    nc.finalize()
    return nc


def _prep_inputs(subgraph, neighs, mask, local_stats, global_stats,
                 emb_table, a_w, a_b, fc1_w, fc1_b, fc2_w, fc2_b):
    """Host-side layout/sharding prep.

    Returns (in_maps, orders, key) where key = (slots, ni_ws, v_ws)."""
    bf = ml_dtypes.bfloat16
    a1 = a_w[0, :H]
    a2 = a_w[0, H:]
    shared = {
        "emb": np.ascontiguousarray(emb_table).astype(bf),
        "a2rep": a2.reshape(1, H).astype(bf),
        "a1rep": a1.reshape(1, H).astype(bf),
        "ab_rep": np.broadcast_to(a_b.astype(np.float32), (TILE, 1)).copy(),
        "ident": np.eye(TILE, dtype=np.float32).astype(bf),
        "w1t_a": np.ascontiguousarray(fc1_w[:, :H].T).astype(bf),
        "w1t_b": np.ascontiguousarray(fc1_w[:, H:].T).astype(bf),
        "b1": fc1_b.reshape(H, 1).astype(np.float32),
        "w2t": np.ascontiguousarray(fc2_w.T).astype(bf),
        "b2row": fc2_b.reshape(1, H).astype(bf),
        "onesc": np.ones((1, TILE), dtype=np.float32).astype(bf),
    }
    keep = mask[:, :, :, 0] < 0.5          # [B,S,N] True = neighbor survives
    counts = 1 + keep.sum(axis=2)          # [B,S]
    orders = np.argsort(-counts, axis=1, kind="stable")

    slots = []
    for t in range(NT):
        c = 0
        for b in range(B):
            c = max(c, int(counts[b, orders[b, t * TILE]]))
        slots.append(c)
    slots = tuple(slots)
    offs = np.concatenate([[0], np.cumsum(slots)]).astype(int)
    ctot = int(offs[-1])

    # absolute slot ids per core (-1 = pad)
    idx_abs_all, padm_all, uniq_all = [], [], []
    for b in range(B):
        order = orders[b]
        idx_abs = np.full((TILE, ctot), -1, dtype=np.int64)
        padm = np.zeros((TILE, ctot), dtype=np.float32)
        for t in range(NT):
            ct = slots[t]
            o0 = offs[t]
            nodes = order[t * TILE:(t + 1) * TILE]
            idx_abs[:, o0] = subgraph[b, nodes]
            for p in range(TILE):
                n = nodes[p]
                kn = neighs[b, n][keep[b, n]]
                idx_abs[p, o0 + 1:o0 + 1 + len(kn)] = kn
                padm[p, o0 + 1 + len(kn):o0 + ct] = 1.0
        idx_abs_all.append(idx_abs)
        padm_all.append(padm)
        uniq_all.append(np.unique(idx_abs[idx_abs >= 0]))

    # per-window unique counts; SPMD-uniform valid counts
    wstarts = []
    for b in range(B):
        ws = np.searchsorted(uniq_all[b], np.arange(NWIN + 1) * WIN)
        wstarts.append(ws)
    v_ws = tuple(int(max(wstarts[b][w + 1] - wstarts[b][w] for b in range(B)))
                 for w in range(NWIN))
    ni_ws = tuple(_rup(v, 16) for v in v_ws)
    blocks_w = [_rup(ni, TILE) // TILE for ni in ni_ws]
    blk_off = np.concatenate([[0], np.cumsum(blocks_w)]).astype(int)
    blk_tot = int(blk_off[-1])
    assert TILE * blk_tot < 32768, f"scratch rows {TILE * blk_tot} exceed int16"

    def wrap16(a):
        return np.ascontiguousarray(a.reshape(-1, 16).T)

    in_maps = []
    for b in range(B):
        uniq = uniq_all[b]
        ws = wstarts[b]
        # phase-1 window index lists
        idx1_parts = []
        row_of_rank = np.empty(len(uniq), dtype=np.int64)
        for w in range(NWIN):
            if ni_ws[w] == 0:
                continue
            rel = uniq[ws[w]:ws[w + 1]] - w * WIN
            nwb = len(rel)
            a = np.full(ni_ws[w], -1, dtype=np.int16)
            a[:nwb] = rel.astype(np.int16)
            a[nwb:v_ws[w]] = 0                     # top-up (dup row) for SPMD
            idx1_parts.append(wrap16(a))
            j = np.arange(nwb)
            row_of_rank[ws[w]:ws[w + 1]] = (j % TILE) * blk_tot + blk_off[w] + j // TILE
        idx1 = np.tile(np.concatenate(idx1_parts, axis=1), (8, 1))

        # phase-2 positional rank lists (slot-major per tile)
        idx_abs = idx_abs_all[b]
        ranks = np.searchsorted(uniq, np.clip(idx_abs, 0, None))
        row2 = row_of_rank[ranks]
        row2[idx_abs < 0] = row_of_rank[0]         # pads -> a known-valid row
        assert row2.max() < 32768
        idx2_parts = []
        for t in range(NT):
            ct = slots[t]
            o0 = offs[t]
            arr = np.ascontiguousarray(row2[:, o0:o0 + ct].T).ravel()  # j = i*128+p
            idx2_parts.append(wrap16(arr.astype(np.int16)))
        idx2 = np.tile(np.concatenate(idx2_parts, axis=1), (8, 1))

        order = orders[b]
        st = np.concatenate(
            [local_stats[b][order].T,
             np.broadcast_to(global_stats[b].reshape(1, 1), (1, S))], axis=0)
        m = dict(shared)
        m.update({
            "idx1": idx1, "idx2": idx2, "padm": padm_all[b],
            "statst": np.ascontiguousarray(st).astype(bf),
        })
        in_maps.append(m)
    return in_maps, orders, (slots, ni_ws, v_ws)


last_exec_ns = None
last_results = None


def kernel(**inputs) -> np.ndarray:
    global last_exec_ns, last_results
    in_maps, orders, key = _prep_inputs(**inputs)
    if key not in _cached:
        _cached[key] = _build_program(*key)
    nc = _cached[key]
    trace = bool(int(os.environ.get("KERNEL_TRACE", "0")))
    res = bass_utils.run_bass_kernel_spmd(
        nc, in_maps, core_ids=list(range(B)), trace=trace)
    last_exec_ns = res.exec_time_ns
    last_results = res
    out = np.empty((B, S, H), dtype=np.float32)
    for b in range(B):
        out[b, orders[b]] = res.results[b]["out"]
    return out


if __name__ == "__main__":
    _build_program((33,) * NT, (5504, 5504, 5504, 448), (5500, 5500, 5500, 440))
    print("program builds OK")
